# revision 9
# baseline (speedup 1.0000x reference)
"""Trainium2 Bass/Tile kernel: MoE-routed per-sample dynamic 3x3 conv (stride 2).

Reference computation:
    pooled  = mean(x, HW)                                        (B, Cin)
    rw      = sigmoid(pooled @ routing_w.T + routing_b)          (B, E)
    kernels = einsum('be,eoihw->boihw', rw, expert_weight)       (B,Cout,Cin,3,3)
    y[b]    = conv2d(x[b], kernels[b], stride 2, pad 1)          (B,Cout,56,56)

Sharding: data-parallel over batch across 8 NeuronCores (4 samples each);
routing/expert weights replicated (host pre-transposes them into the conv
lhsT layout [ci, e, tap, co]).  No collectives.

Precision: the conv datapath runs in bf16 (the PE streams bf16 at 1
col/cycle vs 2 cycles/col for fp32), accumulating in fp32 PSUM.  x and
the expert weights are cast to bf16 on the host, which also halves HBM
read traffic.  The routing chain stays fp32.  Output is written as bf16
and upcast on the host (rel-err budget 2e-2, measured ~4e-3).

Per-core plan (software-pipelined two samples deep):
  - x[b] in SBUF as two bf16 slabs: slab0 = x rows 0..63 (+2 zero pad
    rows), slab1 = x rows 63..111.  Fully contiguous 24-32-row DMA
    descriptors at full HBM rate.
  - global-avg-pool: bf16 tensor_tensor add tree (DVE 2x packed mode;
    tensor_reduce is 1x-only) + small 1x reduces; feeds the routing
    matmul -> sigmoid -> a K=1 ones-matmul broadcast of the gates.
  - combined weights W_b = sum_e rw[b,e]*E_r[e] on DVE, one chunk per
    tap-row (d=1 first, matching conv consumption order).
  - conv: out[co, oh, ow] accumulated over the 9 taps in PSUM, N=448
    matmuls (8 output rows x 56 cols), 7 blocks per (sample, co_tile) in
    two groups: blocks 0-3 read slab0 (output rows 0..31), blocks 4-6
    read slab1 (rows 32..55).  Taps-outer within a group: one LDWEIGHTS
    per (group, tap) serves 3-4 matmuls.  The top pad row (ih=-1) reads
    the zero row; the left pad (iw=-1, dx=0 taps) is handled by skipping
    output column 0 via a strided PSUM out AP (its true contribution is
    the zero pad), so no correction pass is needed.
  - PSUM: group A 4 banks + group B 3 banks + routing 1 bank = 8.  Group
    evictions (ACT copy to a bf16 stage + one contiguous store DMA per
    group) overlap the other group's conv.
"""

import numpy as np

try:
    import concourse.bass as bass
except ImportError:  # toolchain not on sys.path in a fresh interpreter
    import sys

    for _p in ("/opt/trn_rl_repo", "/root/.axon_site/_ro/trn_rl_repo"):
        if _p not in sys.path:
            sys.path.insert(0, _p)
    import concourse.bass as bass

import concourse.mybir as mybir
from concourse.bacc import Bacc
from concourse.bass_utils import run_bass_kernel_spmd
from concourse.tile import TileContext

FP32 = mybir.dt.float32
BF16 = mybir.dt.bfloat16
F8E4 = mybir.dt.float8e4
NP_BF16 = mybir.dt.np(mybir.dt.bfloat16)

N_CORES = 8
B_FULL = 32
B_SH = B_FULL // N_CORES  # 4 samples per core
CIN = 128
H = W = 112
COUT = 256
E = 4
KH = KW = 3
OH = OW = 56
HWSZ = H * W  # 12544
R = 8  # output rows per PSUM block
NN = R * OW  # 448 moving dim per matmul
S_COLS = 112
DROW = 2  # x data starts at this slab row
S0_ROWS = DROW + 64  # slab0: x rows 0..63
S1_ROWS = DROW + 49  # slab1: x rows 63..111
S1_X0 = 63  # first x row held in slab1 (duplicates slab0's last row)

# Tap order matches the weight-combine chunk order (dy=1 first).  The two
# FP8_TAPS are last: for fp8 blocks they are replaced by one DoubleRow pass.
TAPS = [(1, 1), (1, 0), (1, 2), (0, 0), (0, 2), (2, 0), (2, 2), (0, 1), (2, 1)]
# (first block, n blocks) conv groups; group 0 reads slab0, group 1 slab1
GROUPS = ((0, 4), (4, 3))

# fp8 DoubleRow hybrid: taps (0,1) and (2,1) (odd input rows, even cols)
# computed as ONE fp8e4m3 DoubleRow matmul (2 k-tiles) instead of two bf16
# matmuls, for K_FP8 of the 14 (co_tile, block) units per sample.  The PE
# streams a DR pass in the same ~192 ns as one bf16 pass but contracts both
# taps, saving one matmul per fp8 unit.  Quantization noise (measured
# against the bit-exact pipeline model) at K_FP8=12, scale 1.54:
# rel err 1.836e-2 vs the 2e-2 budget (bit-exactly predicted by the
# pipeline model; measured HW deviation ~1e-5 relative).
FP8_TAPS = ((0, 1), (2, 1))
S_FP8 = 1.54  # W *= s, x *= 1/s (reciprocal: product unscaled)
K_FP8 = 14  # fp8 units: ct0 blocks 0..6, ct1 blocks 0..K-8
MROWS, MCOLS = 57, 56  # mini-slab: odd input rows (-1,1,..,111), even cols


def _is_fp8_blk(ct, jb):
    return jb < (7 if ct == 0 else K_FP8 - 7)

_NC_CACHE = {}


def build_nc(rep=1):
    """Build the per-core module.  rep > 1 repeats the whole pipeline (same
    inputs/outputs) — used only for benchmarking slope measurements."""
    if rep in _NC_CACHE:
        return _NC_CACHE[rep]

    # Bacc (not raw Bass): its finalize() runs the legality passes this walrus
    # build needs — move_matmul_waits_to_ldweights + generate_event_semaphores
    # (max 1 sync wait per instruction) + register allocation.
    nc = Bacc(trn_type="TRN2")
    x = nc.dram_tensor("x", [B_SH, CIN, H, W], BF16, kind="ExternalInput")
    # weights arrive pre-transposed from the host (see make_in_maps):
    #   ew_t: [ci, e, tap, co]  (conv lhsT layout, bf16)   rw_t: [ci, e]
    rwt_h = nc.dram_tensor("routing_wt", [CIN, E], BF16, kind="ExternalInput")
    rb_h = nc.dram_tensor("routing_b", [E], FP32, kind="ExternalInput")
    ewt_h = nc.dram_tensor(
        "expert_weight_t", [CIN, E, KH * KW, COUT], BF16, kind="ExternalInput"
    )
    y = nc.dram_tensor("y", [B_SH, COUT, OH, OW], BF16, kind="ExternalOutput")

    with TileContext(nc) as tc:
        with (
            tc.tile_pool(name="const", bufs=1) as const,
            tc.tile_pool(name="slabs", bufs=4) as slabs,
            tc.tile_pool(name="wpool", bufs=4) as wpool,
            tc.tile_pool(name="stage", bufs=2) as stage,
            tc.tile_pool(name="small", bufs=3) as small,
            tc.tile_pool(name="ps_a", bufs=1, space="PSUM") as ps_a,
            tc.tile_pool(name="ps_b", bufs=1, space="PSUM") as ps_b,
            tc.tile_pool(name="ps_rt", bufs=1, space="PSUM") as ps_rt,
        ):
            # ---------------- one-time prep ----------------
            ones_row = const.tile([1, 128], BF16)
            nc.vector.memset(ones_row, 1.0)
            bias_row = const.tile([1, E], FP32)
            nc.sync.dma_start(out=bias_row, in_=rb_h[:].unsqueeze(0))
            rwT = const.tile([128, E], BF16)
            nc.sync.dma_start(out=rwT, in_=rwt_h[:, :])

            # expert weights in lhsT layout [ci, e, tap, co], loaded directly
            e_r = const.tile([128, E, KH * KW, COUT], BF16)

            def emit_expert_prep():
                # tap-group major, d=1 first: the conv consumes d=1 taps first
                for d in (1, 0, 2):
                    nc.sync.dma_start(
                        out=e_r[:, :, 3 * d : 3 * d + 3, :],
                        in_=ewt_h[:, :, 3 * d : 3 * d + 3, :],
                    )

            # ---------------- per-sample pipeline ----------------
            # Emission is software-pipelined two samples deep: sample b+2's
            # loads + routing + weight combine are emitted (= get scheduler
            # priority) before sample b's conv, so they run under the
            # previous convs.
            state = {}
            gstate = {}

            def emit_loads(b):
                slab0 = slabs.tile(
                    [128, S0_ROWS, S_COLS], BF16, tag="s0", name=f"slab0_{b}"
                )
                slab1 = slabs.tile(
                    [128, S1_ROWS, S_COLS], BF16, tag="s1", name=f"slab1_{b}"
                )
                # slab0 rows 0..1: zero pad (the conv's ih=-1 reads row 1).
                # slab1 row 2 duplicates slab0's last row (x row 63) so every
                # conv matmul reads rows from a single slab.
                nc.gpsimd.memset(slab0[:, 0:DROW, :], 0.0)
                nc.sync.dma_start(
                    out=slab0[:, DROW : DROW + 32, :], in_=x[b % B_SH, :, 0:32, :]
                )
                nc.sync.dma_start(
                    out=slab0[:, DROW + 32 : DROW + 64, :],
                    in_=x[b % B_SH, :, 32:64, :],
                )
                nc.sync.dma_start(
                    out=slab1[:, DROW : DROW + 25, :],
                    in_=x[b % B_SH, :, S1_X0 : S1_X0 + 25, :],
                )
                nc.sync.dma_start(
                    out=slab1[:, DROW + 25 : DROW + 49, :],
                    in_=x[b % B_SH, :, S1_X0 + 25 : 112, :],
                )
                # Global-avg-pool via a bf16 tensor_tensor add tree (DVE 2x
                # packed mode; tensor_reduce is 1x-only) + small 1x reduces.
                # Pool rows: slab0[2:66) = x rows 0..63; slab1[3:51) = x rows
                # 64..111 (slab1 row 2 duplicates x row 63 - skip it).
                pooled = small.tile(
                    [128, 3], FP32, tag="pooled", name=f"pooled_{b}"
                )
                ta = small.tile([128, 32 * S_COLS], BF16, tag="ta", name=f"ta_{b}")
                tb = small.tile([128, 24 * S_COLS], BF16, tag="tb", name=f"tb_{b}")
                s0 = slab0[:].rearrange("p a c -> p (a c)")
                s1 = slab1[:].rearrange("p a c -> p (a c)")
                # level-1 adds on GPSIMD (idle engine) to relieve DVE
                nc.gpsimd.tensor_add(
                    out=ta,
                    in0=s0[:, DROW * S_COLS : 34 * S_COLS],
                    in1=s0[:, 34 * S_COLS : 66 * S_COLS],
                )
                nc.gpsimd.tensor_add(
                    out=tb,
                    in0=s1[:, 3 * S_COLS : 27 * S_COLS],
                    in1=s1[:, 27 * S_COLS : 51 * S_COLS],
                )
                h0 = 16 * S_COLS
                nc.vector.tensor_add(
                    out=ta[:, 0:h0], in0=ta[:, 0:h0], in1=ta[:, h0 : 2 * h0]
                )
                h1 = 8 * S_COLS
                nc.vector.tensor_add(
                    out=ta[:, 0:h1], in0=ta[:, 0:h1], in1=ta[:, h1 : 2 * h1]
                )
                h2 = 12 * S_COLS
                nc.vector.tensor_add(
                    out=tb[:, 0:h2], in0=tb[:, 0:h2], in1=tb[:, h2 : 2 * h2]
                )
                h3 = 6 * S_COLS
                nc.vector.tensor_add(
                    out=tb[:, 0:h3], in0=tb[:, 0:h3], in1=tb[:, h3 : 2 * h3]
                )
                nc.vector.tensor_reduce(
                    out=pooled[:, 0:1],
                    in_=ta[:, 0:h1],
                    axis=mybir.AxisListType.X,
                    op=mybir.AluOpType.add,
                )
                nc.vector.tensor_reduce(
                    out=pooled[:, 1:2],
                    in_=tb[:, 0:h3],
                    axis=mybir.AxisListType.X,
                    op=mybir.AluOpType.add,
                )
                nc.vector.tensor_reduce(
                    out=pooled[:, 2:3],
                    in_=pooled[:, 0:2],
                    axis=mybir.AxisListType.X,
                    op=mybir.AluOpType.add,
                )
                gstate[b] = pooled
                state[b] = (slab0, slab1, None)

            def emit_gates(b):
                pooled = gstate.pop(b)
                slab0, slab1, _ = state[b]

                # routing gates (bf16 operands: one PE pass vs fp32's
                # LOW+HIGH double pass; precision is ample for the logits)
                pool_bf = small.tile([128, 1], BF16, tag="poolbf", name=f"pbf_{b}")
                nc.scalar.copy(out=pool_bf, in_=pooled[:, 2:3])
                lg_ps = ps_rt.tile([1, E], FP32, tag="pr", name=f"lg_{b}")
                nc.tensor.matmul(lg_ps, pool_bf, rwT, start=True, stop=True)
                lg_sb = small.tile([1, E], FP32, tag="lg", name=f"lgs_{b}")
                nc.vector.scalar_tensor_tensor(
                    out=lg_sb,
                    in0=lg_ps,
                    scalar=1.0 / HWSZ,
                    in1=bias_row,
                    op0=mybir.AluOpType.mult,
                    op1=mybir.AluOpType.add,
                )
                sig = small.tile([1, E], BF16, tag="sig", name=f"sig_{b}")
                nc.scalar.activation(
                    out=sig, in_=lg_sb, func=mybir.ActivationFunctionType.Sigmoid
                )
                bc_ps = ps_rt.tile([128, E], FP32, tag="pr", name=f"bc_{b}")
                nc.tensor.matmul(bc_ps, ones_row, sig, start=True, stop=True)
                rw_sb = small.tile([128, E], FP32, tag="rws", name=f"rws_{b}")
                nc.scalar.copy(out=rw_sb, in_=bc_ps)

                # combined per-sample conv weights, one contiguous chunk per
                # tap-row (768 bf16 elements), d=1 first to match the conv.
                # single-chunk combine (one TS + 3 STT over all 9 taps):
                # same per-element numerics as chunked, fewer DVE dispatches
                wb = wpool.tile([128, KH * KW, COUT], BF16, tag="wb", name=f"wb_{b}")
                dstf = wb[:].rearrange("p a c -> p (a c)")
                srcs = [
                    e_r[:, e, :, :].rearrange("p a c -> p (a c)") for e in range(E)
                ]
                nc.vector.tensor_scalar_mul(
                    out=dstf,
                    in0=srcs[0],
                    scalar1=rw_sb[:, 0:1],
                )
                for e in range(1, E):
                    nc.vector.scalar_tensor_tensor(
                        out=dstf,
                        in0=srcs[e],
                        scalar=rw_sb[:, e : e + 1],
                        in1=dstf,
                        op0=mybir.AluOpType.mult,
                        op1=mybir.AluOpType.add,
                    )

                # fp8 DoubleRow operands: weight pair [ci, 2, co] = s*wb taps
                # (0,1),(2,1); mini-slab [ci, 57, 56] = x/s at odd input rows
                # (row 0 = ih=-1 zero pad), even cols.
                w8 = wpool.tile([128, 2, COUT], F8E4, tag="w8", name=f"w8_{b}")
                for k, (dy, dx) in enumerate(FP8_TAPS):
                    nc.vector.tensor_scalar_mul(
                        out=w8[:, k, :],
                        in0=wb[:, 3 * dy + dx, :],
                        scalar1=float(S_FP8),
                    )
                m8 = wpool.tile([128, MROWS, MCOLS], F8E4, tag="m8", name=f"m8_{b}")
                nc.gpsimd.memset(m8[:, 0:1, :], 0.0)
                s0v, s1v = slab0[:], slab1[:]
                # odd x rows 1..63 from slab0 (rows DROW+1, step 2), even cols
                src0 = bass.AP(
                    tensor=s0v.tensor,
                    offset=s0v.offset + (DROW + 1) * S_COLS,
                    ap=[[s0v.ap[0][0], 128], [2 * S_COLS, 32], [1, MCOLS]],
                )
                nc.scalar.activation(
                    out=m8[:, 1:33, :],
                    in_=src0,
                    func=mybir.ActivationFunctionType.Identity,
                    scale=1.0 / S_FP8,
                )
                # odd x rows 65..111 from slab1 (rows DROW+2, step 2)
                src1 = bass.AP(
                    tensor=s1v.tensor,
                    offset=s1v.offset + (DROW + 2) * S_COLS,
                    ap=[[s1v.ap[0][0], 128], [2 * S_COLS, 24], [1, MCOLS]],
                )
                nc.scalar.activation(
                    out=m8[:, 33:57, :],
                    in_=src1,
                    func=mybir.ActivationFunctionType.Identity,
                    scale=1.0 / S_FP8,
                )
                state[b] = (slab0, slab1, wb, w8, m8)

            def emit_conv(b):
                slab0, slab1, wb, w8, m8 = state.pop(b)
                m8v = m8[:]
                for ct in range(2):
                    for gi, (j0, ng) in enumerate(GROUPS):
                        sl = slab0 if gi == 0 else slab1
                        fv = sl[:]
                        pool = ps_a if gi == 0 else ps_b
                        ps = pool.tile(
                            [128, ng, 512], FP32, tag="pc", name=f"ps_{b}_{ct}_{gi}"
                        )
                        for ti, (dy, dx) in enumerate(TAPS):
                            lhsT = wb[:, dy * 3 + dx, ct * 128 : (ct + 1) * 128]
                            for j in range(ng):
                                jb = j0 + j
                                if (dy, dx) in FP8_TAPS and _is_fp8_blk(ct, jb):
                                    continue
                                # slab row of input row ih = 16*jb + 2r + dy - 1
                                sr = DROW + 16 * jb + dy - 1
                                if gi == 1:
                                    sr -= S1_X0
                                if dx == 0:
                                    # left pad: ow=0 would read iw=-1 (zero
                                    # pad).  Skip output column 0 instead:
                                    # strided PSUM out AP.  Odd col idx ow-1
                                    # -> slab cols 56..110.
                                    rhs = bass.AP(
                                        tensor=fv.tensor,
                                        offset=fv.offset + sr * S_COLS + OW,
                                        ap=[
                                            [fv.ap[0][0], 128],
                                            [2 * S_COLS, R],
                                            [1, OW - 1],
                                        ],
                                    )
                                    out_ap = ps[:, j, 0:NN].rearrange(
                                        "p (r c) -> p r c", c=OW
                                    )[:, :, 1:OW]
                                else:
                                    # dx=1: even cols 0..55; dx=2: odd cols
                                    # ow -> slab cols 56..111
                                    coff = 0 if dx == 1 else OW
                                    rhs = bass.AP(
                                        tensor=fv.tensor,
                                        offset=fv.offset + sr * S_COLS + coff,
                                        ap=[
                                            [fv.ap[0][0], 128],
                                            [2 * S_COLS, R],
                                            [1, OW],
                                        ],
                                    )
                                    out_ap = ps[:, j, 0:NN]
                                nc.tensor.matmul(
                                    out_ap,
                                    lhsT,
                                    rhs,
                                    start=(ti == 0),
                                    stop=(ti == KH * KW - 1),
                                )
                        # fp8 DoubleRow pass: both FP8_TAPS in one matmul per
                        # fp8 block (k-tile stride = one mini-slab row)
                        lhsT8 = w8[:, :, ct * 128 : (ct + 1) * 128]
                        for j in range(ng):
                            jb = j0 + j
                            if not _is_fp8_blk(ct, jb):
                                continue
                            rhs8 = bass.AP(
                                tensor=m8v.tensor,
                                offset=m8v.offset + jb * 8 * MCOLS,
                                ap=[[m8v.ap[0][0], 128], [MCOLS, 2], [1, NN]],
                            )
                            nc.tensor.matmul(
                                ps[:, j, 0:NN],
                                lhsT8,
                                rhs8,
                                start=False,
                                stop=True,
                                perf_mode=mybir.MatmulPerfMode.DoubleRow,
                            )
                        # evict group to a bf16 stage, one contiguous store
                        st = stage.tile(
                            [128, ng * NN],
                            BF16,
                            tag=f"st{gi}",
                            name=f"st_{b}_{ct}_{gi}",
                        )
                        nc.scalar.copy(
                            out=st.rearrange("p (a c) -> p a c", c=NN),
                            in_=ps[:, 0:ng, 0:NN],
                        )
                        yv = y[b % B_SH, ct * 128 : (ct + 1) * 128, :, :].rearrange(
                            "p a c -> p (a c)"
                        )
                        nc.sync.dma_start(
                            out=yv[:, j0 * NN : (j0 + ng) * NN],
                            in_=st,
                        )

            nb = B_SH * rep
            emit_loads(0)
            emit_expert_prep()
            emit_gates(0)
            if nb > 1:
                emit_loads(1)
                emit_gates(1)
            for b in range(nb):
                if b + 2 < nb:
                    emit_loads(b + 2)
                    emit_gates(b + 2)
                emit_conv(b)

    nc.finalize()
    _NC_CACHE[rep] = nc
    return nc


def make_in_maps(x, routing_w, routing_b, expert_weight):
    x = np.asarray(x, dtype=np.float32)
    routing_w = np.asarray(routing_w, dtype=np.float32)
    routing_b = np.ascontiguousarray(np.asarray(routing_b, dtype=np.float32))
    expert_weight = np.asarray(expert_weight, dtype=np.float32)
    # host-side relayout + bf16 cast (replicated across cores):
    #   expert_weight [e, co, ci, kh, kw] -> [ci, e, kh*kw, co]
    ew_t = np.ascontiguousarray(
        expert_weight.transpose(2, 0, 3, 4, 1)
        .reshape(CIN, E, KH * KW, COUT)
        .astype(NP_BF16)
    )
    rw_t = np.ascontiguousarray(routing_w.T.astype(NP_BF16))
    # de-interleave W into [even cols | odd cols] per row: every conv tap
    # then reads a stride-1 run (even taps cols 0..55, odd taps 56..111)
    x_bf = x.astype(NP_BF16)
    x_bf = np.ascontiguousarray(
        np.concatenate([x_bf[..., 0::2], x_bf[..., 1::2]], axis=-1)
    )
    return [
        {
            "x": np.ascontiguousarray(x_bf[c * B_SH : (c + 1) * B_SH]),
            "routing_wt": rw_t,
            "routing_b": routing_b,
            "expert_weight_t": ew_t,
        }
        for c in range(N_CORES)
    ]


def kernel(x, routing_w, routing_b, expert_weight):
    nc = build_nc()
    in_maps = make_in_maps(x, routing_w, routing_b, expert_weight)
    res = run_bass_kernel_spmd(nc, in_maps, core_ids=list(range(N_CORES)))
    return np.concatenate(
        [res.results[c]["y"] for c in range(N_CORES)], axis=0
    ).astype(np.float32)



# revision 11
# speedup vs baseline: 1.2959x; 1.2959x over previous
"""Trainium2 Bass/Tile kernel: MoE-routed per-sample dynamic 3x3 conv (stride 2).

Reference computation:
    pooled  = mean(x, HW)                                        (B, Cin)
    rw      = sigmoid(pooled @ routing_w.T + routing_b)          (B, E)
    kernels = einsum('be,eoihw->boihw', rw, expert_weight)       (B,Cout,Cin,3,3)
    y[b]    = conv2d(x[b], kernels[b], stride 2, pad 1)          (B,Cout,56,56)

Sharding: data-parallel over batch across 8 NeuronCores (4 samples each);
routing/expert weights replicated (host pre-transposes them into the conv
lhsT layout [ci, e, tap, co]).  No collectives.

Precision: the conv datapath runs in bf16 (the PE streams bf16 at 1
col/cycle vs 2 cycles/col for fp32), accumulating in fp32 PSUM.  x and
the expert weights are cast to bf16 on the host, which also halves HBM
read traffic.  The routing chain stays fp32.  Output is written as bf16
and upcast on the host (rel-err budget 2e-2, measured ~4e-3).

Per-core plan (software-pipelined two samples deep):
  - x[b] in SBUF as two bf16 slabs: slab0 = x rows 0..63 (+2 zero pad
    rows), slab1 = x rows 63..111.  Fully contiguous 24-32-row DMA
    descriptors at full HBM rate.
  - global-avg-pool: bf16 tensor_tensor add tree (DVE 2x packed mode;
    tensor_reduce is 1x-only) + small 1x reduces; feeds the routing
    matmul -> sigmoid -> a K=1 ones-matmul broadcast of the gates.
  - combined weights W_b = sum_e rw[b,e]*E_r[e] on DVE, one chunk per
    tap-row (d=1 first, matching conv consumption order).
  - conv: out[co, oh, ow] accumulated over the 9 taps in PSUM, N=448
    matmuls (8 output rows x 56 cols), 7 blocks per (sample, co_tile) in
    two groups: blocks 0-3 read slab0 (output rows 0..31), blocks 4-6
    read slab1 (rows 32..55).  Taps-outer within a group: one LDWEIGHTS
    per (group, tap) serves 3-4 matmuls.  The top pad row (ih=-1) reads
    the zero row; the left pad (iw=-1, dx=0 taps) is handled by skipping
    output column 0 via a strided PSUM out AP (its true contribution is
    the zero pad), so no correction pass is needed.
  - PSUM: group A 4 banks + group B 3 banks + routing 1 bank = 8.  Group
    evictions (ACT copy to a bf16 stage + one contiguous store DMA per
    group) overlap the other group's conv.
"""

import numpy as np

try:
    import concourse.bass as bass
except ImportError:  # toolchain not on sys.path in a fresh interpreter
    import sys

    for _p in ("/opt/trn_rl_repo", "/root/.axon_site/_ro/trn_rl_repo"):
        if _p not in sys.path:
            sys.path.insert(0, _p)
    import concourse.bass as bass

import concourse.mybir as mybir
from concourse.bacc import Bacc
from concourse.bass_utils import run_bass_kernel_spmd
from concourse.tile import TileContext

FP32 = mybir.dt.float32
BF16 = mybir.dt.bfloat16
F8E4 = mybir.dt.float8e4
NP_BF16 = mybir.dt.np(mybir.dt.bfloat16)

N_CORES = 8
B_FULL = 32
B_SH = B_FULL // N_CORES  # 4 samples per core
CIN = 128
H = W = 112
COUT = 256
E = 4
KH = KW = 3
OH = OW = 56
HWSZ = H * W  # 12544
R = 8  # output rows per PSUM block
NN = R * OW  # 448 moving dim per matmul
S_COLS = 112
DROW = 2  # x data starts at this slab row
S0_ROWS = DROW + 64  # slab0: x rows 0..63
S1_ROWS = DROW + 49  # slab1: x rows 63..111
S1_X0 = 63  # first x row held in slab1 (duplicates slab0's last row)

# Tap order matches the weight-combine chunk order (dy=1 first).  The two
# FP8_TAPS are last: for fp8 blocks they are replaced by one DoubleRow pass.
TAPS = [(1, 1), (1, 0), (1, 2), (0, 0), (0, 2), (2, 0), (2, 2), (0, 1), (2, 1)]
# (first block, n blocks) conv groups; group 0 reads slab0, group 1 slab1
GROUPS = ((0, 4), (4, 3))

# fp8 DoubleRow hybrid: taps (0,1) and (2,1) (odd input rows, even cols)
# computed as ONE fp8e4m3 DoubleRow matmul (2 k-tiles) instead of two bf16
# matmuls, for K_FP8 of the 14 (co_tile, block) units per sample.  The PE
# streams a DR pass in the same ~192 ns as one bf16 pass but contracts both
# taps, saving one matmul per fp8 unit.  Quantization noise (measured
# against the bit-exact pipeline model) at K_FP8=12, scale 1.54:
# rel err 1.836e-2 vs the 2e-2 budget (bit-exactly predicted by the
# pipeline model; measured HW deviation ~1e-5 relative).
FP8_TAPS = ((0, 1), (2, 1))
S_FP8 = 1.54  # W *= s, x *= 1/s (reciprocal: product unscaled)
K_FP8 = 14  # fp8 units: ct0 blocks 0..6, ct1 blocks 0..K-8
MROWS, MCOLS = 57, 56  # mini-slab: odd input rows (-1,1,..,111), even cols


def _is_fp8_blk(ct, jb):
    return jb < (7 if ct == 0 else K_FP8 - 7)

_NC_CACHE = {}


def build_nc(rep=1):
    """Build the per-core module.  rep > 1 repeats the whole pipeline (same
    inputs/outputs) — used only for benchmarking slope measurements."""
    if rep in _NC_CACHE:
        return _NC_CACHE[rep]

    # Bacc (not raw Bass): its finalize() runs the legality passes this walrus
    # build needs — move_matmul_waits_to_ldweights + generate_event_semaphores
    # (max 1 sync wait per instruction) + register allocation.
    nc = Bacc(trn_type="TRN2")
    x = nc.dram_tensor("x", [B_SH, CIN, H, W], BF16, kind="ExternalInput")
    # weights arrive pre-transposed from the host (see make_in_maps):
    #   ew_t: [ci, e, tap, co]  (conv lhsT layout, bf16)   rw_t: [ci, e]
    rwt_h = nc.dram_tensor("routing_wt", [CIN, E], BF16, kind="ExternalInput")
    rb_h = nc.dram_tensor("routing_b", [E], FP32, kind="ExternalInput")
    ewt_h = nc.dram_tensor(
        "expert_weight_t", [CIN, E, KH * KW, COUT], BF16, kind="ExternalInput"
    )
    y = nc.dram_tensor("y", [B_SH, COUT, OH, OW], BF16, kind="ExternalOutput")

    with TileContext(nc) as tc:
        with (
            tc.tile_pool(name="const", bufs=1) as const,
            tc.tile_pool(name="slabs", bufs=4) as slabs,
            tc.tile_pool(name="wpool", bufs=4) as wpool,
            tc.tile_pool(name="stage", bufs=2) as stage,
            tc.tile_pool(name="small", bufs=3) as small,
            tc.tile_pool(name="ps_a", bufs=1, space="PSUM") as ps_a,
            tc.tile_pool(name="ps_b", bufs=1, space="PSUM") as ps_b,
            tc.tile_pool(name="ps_rt", bufs=1, space="PSUM") as ps_rt,
        ):
            # ---------------- one-time prep ----------------
            ones_row = const.tile([1, 128], BF16)
            nc.vector.memset(ones_row, 1.0)
            bias_row = const.tile([1, E], FP32)
            nc.sync.dma_start(out=bias_row, in_=rb_h[:].unsqueeze(0))
            rwT = const.tile([128, E], BF16)
            nc.sync.dma_start(out=rwT, in_=rwt_h[:, :])

            # expert weights in lhsT layout [ci, e, tap, co], loaded directly
            e_r = const.tile([128, E, KH * KW, COUT], BF16)

            def emit_expert_prep():
                # tap-group major, d=1 first: the conv consumes d=1 taps first
                for d in (1, 0, 2):
                    nc.sync.dma_start(
                        out=e_r[:, :, 3 * d : 3 * d + 3, :],
                        in_=ewt_h[:, :, 3 * d : 3 * d + 3, :],
                    )

            # ---------------- per-sample pipeline ----------------
            # Emission is software-pipelined two samples deep: sample b+2's
            # loads + routing + weight combine are emitted (= get scheduler
            # priority) before sample b's conv, so they run under the
            # previous convs.
            state = {}
            gstate = {}

            def emit_loads(b):
                slab0 = slabs.tile(
                    [128, S0_ROWS, S_COLS], BF16, tag="s0", name=f"slab0_{b}"
                )
                slab1 = slabs.tile(
                    [128, S1_ROWS, S_COLS], BF16, tag="s1", name=f"slab1_{b}"
                )
                # slab0 rows 0..1: zero pad (the conv's ih=-1 reads row 1).
                # slab1 row 2 duplicates slab0's last row (x row 63) so every
                # conv matmul reads rows from a single slab.
                nc.gpsimd.memset(slab0[:, 0:DROW, :], 0.0)
                nc.sync.dma_start(
                    out=slab0[:, DROW : DROW + 32, :], in_=x[b % B_SH, :, 0:32, :]
                )
                nc.sync.dma_start(
                    out=slab0[:, DROW + 32 : DROW + 64, :],
                    in_=x[b % B_SH, :, 32:64, :],
                )
                nc.sync.dma_start(
                    out=slab1[:, DROW : DROW + 25, :],
                    in_=x[b % B_SH, :, S1_X0 : S1_X0 + 25, :],
                )
                nc.sync.dma_start(
                    out=slab1[:, DROW + 25 : DROW + 49, :],
                    in_=x[b % B_SH, :, S1_X0 + 25 : 112, :],
                )
                # Global-avg-pool via a bf16 tensor_tensor add tree (DVE 2x
                # packed mode; tensor_reduce is 1x-only) + small 1x reduces.
                # Pool rows: slab0[2:66) = x rows 0..63; slab1[3:51) = x rows
                # 64..111 (slab1 row 2 duplicates x row 63 - skip it).
                pooled = small.tile(
                    [128, 3], FP32, tag="pooled", name=f"pooled_{b}"
                )
                ta = small.tile([128, 32 * S_COLS], BF16, tag="ta", name=f"ta_{b}")
                tb = small.tile([128, 24 * S_COLS], BF16, tag="tb", name=f"tb_{b}")
                s0 = slab0[:].rearrange("p a c -> p (a c)")
                s1 = slab1[:].rearrange("p a c -> p (a c)")
                nc.vector.tensor_add(
                    out=ta,
                    in0=s0[:, DROW * S_COLS : 34 * S_COLS],
                    in1=s0[:, 34 * S_COLS : 66 * S_COLS],
                )
                nc.vector.tensor_add(
                    out=tb,
                    in0=s1[:, 3 * S_COLS : 27 * S_COLS],
                    in1=s1[:, 27 * S_COLS : 51 * S_COLS],
                )
                h0 = 16 * S_COLS
                nc.vector.tensor_add(
                    out=ta[:, 0:h0], in0=ta[:, 0:h0], in1=ta[:, h0 : 2 * h0]
                )
                h1 = 8 * S_COLS
                nc.vector.tensor_add(
                    out=ta[:, 0:h1], in0=ta[:, 0:h1], in1=ta[:, h1 : 2 * h1]
                )
                h2 = 12 * S_COLS
                nc.vector.tensor_add(
                    out=tb[:, 0:h2], in0=tb[:, 0:h2], in1=tb[:, h2 : 2 * h2]
                )
                h3 = 6 * S_COLS
                nc.vector.tensor_add(
                    out=tb[:, 0:h3], in0=tb[:, 0:h3], in1=tb[:, h3 : 2 * h3]
                )
                nc.vector.tensor_reduce(
                    out=pooled[:, 0:1],
                    in_=ta[:, 0:h1],
                    axis=mybir.AxisListType.X,
                    op=mybir.AluOpType.add,
                )
                nc.vector.tensor_reduce(
                    out=pooled[:, 1:2],
                    in_=tb[:, 0:h3],
                    axis=mybir.AxisListType.X,
                    op=mybir.AluOpType.add,
                )
                nc.vector.tensor_reduce(
                    out=pooled[:, 2:3],
                    in_=pooled[:, 0:2],
                    axis=mybir.AxisListType.X,
                    op=mybir.AluOpType.add,
                )
                gstate[b] = pooled
                state[b] = (slab0, slab1, None)

            def emit_gates(b):
                pooled = gstate.pop(b)
                slab0, slab1, _ = state[b]

                # routing gates (bf16 operands: one PE pass vs fp32's
                # LOW+HIGH double pass; precision is ample for the logits)
                pool_bf = small.tile([128, 1], BF16, tag="poolbf", name=f"pbf_{b}")
                nc.scalar.copy(out=pool_bf, in_=pooled[:, 2:3])
                lg_ps = ps_rt.tile([1, E], FP32, tag="pr", name=f"lg_{b}")
                nc.tensor.matmul(lg_ps, pool_bf, rwT, start=True, stop=True)
                lg_sb = small.tile([1, E], FP32, tag="lg", name=f"lgs_{b}")
                nc.vector.scalar_tensor_tensor(
                    out=lg_sb,
                    in0=lg_ps,
                    scalar=1.0 / HWSZ,
                    in1=bias_row,
                    op0=mybir.AluOpType.mult,
                    op1=mybir.AluOpType.add,
                )
                sig = small.tile([1, E], BF16, tag="sig", name=f"sig_{b}")
                nc.scalar.activation(
                    out=sig, in_=lg_sb, func=mybir.ActivationFunctionType.Sigmoid
                )
                bc_ps = ps_rt.tile([128, E], FP32, tag="pr", name=f"bc_{b}")
                nc.tensor.matmul(bc_ps, ones_row, sig, start=True, stop=True)
                rw_sb = small.tile([128, E], FP32, tag="rws", name=f"rws_{b}")
                nc.scalar.copy(out=rw_sb, in_=bc_ps)

                # combined per-sample conv weights, one contiguous chunk per
                # tap-row (768 bf16 elements), d=1 first to match the conv.
                wb = wpool.tile([128, KH * KW, COUT], BF16, tag="wb", name=f"wb_{b}")
                for d in (1, 0, 2):
                    dstf = wb[:, 3 * d : 3 * d + 3, :].rearrange("p a c -> p (a c)")
                    srcs = [
                        e_r[:, e, 3 * d : 3 * d + 3, :].rearrange("p a c -> p (a c)")
                        for e in range(E)
                    ]
                    nc.vector.tensor_scalar_mul(
                        out=dstf,
                        in0=srcs[0],
                        scalar1=rw_sb[:, 0:1],
                    )
                    for e in range(1, E):
                        nc.vector.scalar_tensor_tensor(
                            out=dstf,
                            in0=srcs[e],
                            scalar=rw_sb[:, e : e + 1],
                            in1=dstf,
                            op0=mybir.AluOpType.mult,
                            op1=mybir.AluOpType.add,
                        )

                # fp8 DoubleRow operands: weight pair [ci, 2, co] = s*wb taps
                # (0,1),(2,1); mini-slab [ci, 57, 56] = x/s at odd input rows
                # (row 0 = ih=-1 zero pad), even cols.
                w8 = wpool.tile([128, 2, COUT], F8E4, tag="w8", name=f"w8_{b}")
                for k, (dy, dx) in enumerate(FP8_TAPS):
                    nc.vector.tensor_scalar_mul(
                        out=w8[:, k, :],
                        in0=wb[:, 3 * dy + dx, :],
                        scalar1=float(S_FP8),
                    )
                m8 = wpool.tile([128, MROWS, MCOLS], F8E4, tag="m8", name=f"m8_{b}")
                nc.gpsimd.memset(m8[:, 0:1, :], 0.0)
                s0v, s1v = slab0[:], slab1[:]
                # odd x rows 1..63 from slab0 (rows DROW+1, step 2), even cols
                src0 = bass.AP(
                    tensor=s0v.tensor,
                    offset=s0v.offset + (DROW + 1) * S_COLS,
                    ap=[[s0v.ap[0][0], 128], [2 * S_COLS, 32], [1, MCOLS]],
                )
                nc.scalar.activation(
                    out=m8[:, 1:33, :],
                    in_=src0,
                    func=mybir.ActivationFunctionType.Identity,
                    scale=1.0 / S_FP8,
                )
                # odd x rows 65..111 from slab1 (rows DROW+2, step 2)
                src1 = bass.AP(
                    tensor=s1v.tensor,
                    offset=s1v.offset + (DROW + 2) * S_COLS,
                    ap=[[s1v.ap[0][0], 128], [2 * S_COLS, 24], [1, MCOLS]],
                )
                nc.scalar.activation(
                    out=m8[:, 33:57, :],
                    in_=src1,
                    func=mybir.ActivationFunctionType.Identity,
                    scale=1.0 / S_FP8,
                )
                state[b] = (slab0, slab1, wb, w8, m8)

            def emit_conv(b):
                slab0, slab1, wb, w8, m8 = state.pop(b)
                m8v = m8[:]
                for ct in range(2):
                    for gi, (j0, ng) in enumerate(GROUPS):
                        sl = slab0 if gi == 0 else slab1
                        fv = sl[:]
                        pool = ps_a if gi == 0 else ps_b
                        ps = pool.tile(
                            [128, ng, 512], FP32, tag="pc", name=f"ps_{b}_{ct}_{gi}"
                        )
                        for ti, (dy, dx) in enumerate(TAPS):
                            lhsT = wb[:, dy * 3 + dx, ct * 128 : (ct + 1) * 128]
                            for j in range(ng):
                                jb = j0 + j
                                if (dy, dx) in FP8_TAPS and _is_fp8_blk(ct, jb):
                                    continue
                                # slab row of input row ih = 16*jb + 2r + dy - 1
                                sr = DROW + 16 * jb + dy - 1
                                if gi == 1:
                                    sr -= S1_X0
                                if dx == 0:
                                    # left pad: ow=0 would read iw=-1 (zero
                                    # pad).  Skip output column 0 instead:
                                    # strided PSUM out AP.  Odd col idx ow-1
                                    # -> slab cols 56..110.
                                    rhs = bass.AP(
                                        tensor=fv.tensor,
                                        offset=fv.offset + sr * S_COLS + OW,
                                        ap=[
                                            [fv.ap[0][0], 128],
                                            [2 * S_COLS, R],
                                            [1, OW - 1],
                                        ],
                                    )
                                    out_ap = ps[:, j, 0:NN].rearrange(
                                        "p (r c) -> p r c", c=OW
                                    )[:, :, 1:OW]
                                else:
                                    # dx=1: even cols 0..55; dx=2: odd cols
                                    # ow -> slab cols 56..111
                                    coff = 0 if dx == 1 else OW
                                    rhs = bass.AP(
                                        tensor=fv.tensor,
                                        offset=fv.offset + sr * S_COLS + coff,
                                        ap=[
                                            [fv.ap[0][0], 128],
                                            [2 * S_COLS, R],
                                            [1, OW],
                                        ],
                                    )
                                    out_ap = ps[:, j, 0:NN]
                                nc.tensor.matmul(
                                    out_ap,
                                    lhsT,
                                    rhs,
                                    start=(ti == 0),
                                    stop=(ti == KH * KW - 1),
                                )
                        # fp8 DoubleRow pass: both FP8_TAPS in one matmul per
                        # fp8 block (k-tile stride = one mini-slab row)
                        lhsT8 = w8[:, :, ct * 128 : (ct + 1) * 128]
                        for j in range(ng):
                            jb = j0 + j
                            if not _is_fp8_blk(ct, jb):
                                continue
                            rhs8 = bass.AP(
                                tensor=m8v.tensor,
                                offset=m8v.offset + jb * 8 * MCOLS,
                                ap=[[m8v.ap[0][0], 128], [MCOLS, 2], [1, NN]],
                            )
                            nc.tensor.matmul(
                                ps[:, j, 0:NN],
                                lhsT8,
                                rhs8,
                                start=False,
                                stop=True,
                                perf_mode=mybir.MatmulPerfMode.DoubleRow,
                            )
                        # evict group to a bf16 stage, one contiguous store
                        st = stage.tile(
                            [128, ng * NN],
                            BF16,
                            tag=f"st{gi}",
                            name=f"st_{b}_{ct}_{gi}",
                        )
                        nc.scalar.copy(
                            out=st.rearrange("p (a c) -> p a c", c=NN),
                            in_=ps[:, 0:ng, 0:NN],
                        )
                        yv = y[b % B_SH, ct * 128 : (ct + 1) * 128, :, :].rearrange(
                            "p a c -> p (a c)"
                        )
                        nc.sync.dma_start(
                            out=yv[:, j0 * NN : (j0 + ng) * NN],
                            in_=st,
                        )

            nb = B_SH * rep
            emit_loads(0)
            emit_expert_prep()
            emit_gates(0)
            if nb > 1:
                emit_loads(1)
                emit_gates(1)
            for b in range(nb):
                if b + 2 < nb:
                    emit_loads(b + 2)
                    emit_gates(b + 2)
                emit_conv(b)

    nc.finalize()
    _NC_CACHE[rep] = nc
    return nc


def make_in_maps(x, routing_w, routing_b, expert_weight):
    x = np.asarray(x, dtype=np.float32)
    routing_w = np.asarray(routing_w, dtype=np.float32)
    routing_b = np.ascontiguousarray(np.asarray(routing_b, dtype=np.float32))
    expert_weight = np.asarray(expert_weight, dtype=np.float32)
    # host-side relayout + bf16 cast (replicated across cores):
    #   expert_weight [e, co, ci, kh, kw] -> [ci, e, kh*kw, co]
    ew_t = np.ascontiguousarray(
        expert_weight.transpose(2, 0, 3, 4, 1)
        .reshape(CIN, E, KH * KW, COUT)
        .astype(NP_BF16)
    )
    rw_t = np.ascontiguousarray(routing_w.T.astype(NP_BF16))
    # de-interleave W into [even cols | odd cols] per row: every conv tap
    # then reads a stride-1 run (even taps cols 0..55, odd taps 56..111)
    x_bf = x.astype(NP_BF16)
    x_bf = np.ascontiguousarray(
        np.concatenate([x_bf[..., 0::2], x_bf[..., 1::2]], axis=-1)
    )
    return [
        {
            "x": np.ascontiguousarray(x_bf[c * B_SH : (c + 1) * B_SH]),
            "routing_wt": rw_t,
            "routing_b": routing_b,
            "expert_weight_t": ew_t,
        }
        for c in range(N_CORES)
    ]


def kernel(x, routing_w, routing_b, expert_weight):
    nc = build_nc()
    in_maps = make_in_maps(x, routing_w, routing_b, expert_weight)
    res = run_bass_kernel_spmd(nc, in_maps, core_ids=list(range(N_CORES)))
    return np.concatenate(
        [res.results[c]["y"] for c in range(N_CORES)], axis=0
    ).astype(np.float32)

